# revision 1
# baseline (speedup 1.0000x reference)
"""Expert-parallel MoE ConditionalFeedForward (SwiGLU) for 8 Trainium2 cores.

Math (per token t, selected expert e):
    out[t] = (silu(x[t] @ w1[e].T) * (x[t] @ w3[e].T)) @ w2[e]

Strategy: one expert per NeuronCore (8 experts / 8 cores). The host routes
tokens to experts (gather), each core runs the dense SwiGLU FFN for its
expert's tokens, and the host scatters results back into [T, top_k, D].

On-chip layout keeps the hidden dim H on SBUF partitions throughout
([h, tok] activations), so stage-1 outputs feed stage-2 matmuls with no
transposes. All matmul operands are float32r (full-rate fp32-reduced).
"""

import numpy as np

import concourse.bacc as bacc
import concourse.mybir as mybir
from concourse.bass_utils import run_bass_kernel_spmd
from concourse.tile import TileContext

# Problem constants (nn_ConditionalFeedForward: dim=1024, hidden=2816, 8 experts, top-2)
T = 2048
D = 1024
H = 2816
E = 8
TOPK = 2
ND = D // 128   # 8 d-chunks
NH = H // 128   # 22 h-tiles

F32 = mybir.dt.float32
F32R = mybir.dt.float32r

_BUILD_CACHE: dict[tuple, object] = {}


def _build(npad: int, loop_n: int = 0):
    """Bass program for one core: dense SwiGLU FFN over npad tokens.

    loop_n > 0 wraps the body in a hardware loop (benchmarking only).
    """
    key = (npad, loop_n)
    if key in _BUILD_CACHE:
        return _BUILD_CACHE[key]
    # token chunks: as few as possible, each <=512 (one PSUM bank) and
    # >=256 so float32r matmuls run at full rate
    nchunks = -(-npad // 512)
    base = npad // nchunks
    sizes = [base + (1 if i < npad % nchunks else 0) for i in range(nchunks)]
    chunks, off = [], 0
    for sz in sizes:
        chunks.append((off, sz))
        off += sz

    nc = bacc.Bacc("TRN2", target_bir_lowering=False)
    xt = nc.dram_tensor("xt", [128, ND * npad], F32R, kind="ExternalInput")
    w13 = nc.dram_tensor("w13", [NH, 128, 2 * ND * 128], F32R, kind="ExternalInput")
    w2t = nc.dram_tensor("w2t", [ND, 128, NH * 128], F32R, kind="ExternalInput")
    outt = nc.dram_tensor("outt", [ND, 128, npad], F32, kind="ExternalOutput")

    import contextlib

    with TileContext(nc) as tc:
        with (
            tc.For_i(0, loop_n, 1) if loop_n else contextlib.nullcontext(),
            tc.tile_pool(name="xg", bufs=1) as xg_pool,
            tc.tile_pool(name="w13p", bufs=6) as w13_pool,
            tc.tile_pool(name="w2p", bufs=3) as w2_pool,
            tc.tile_pool(name="tmp", bufs=4) as tmp_pool,
        ):
            x_sb = xg_pool.tile([128, ND * npad], F32R)
            w13_first = w13_pool.tile([128, 2 * ND * 128], F32R, tag="wt", name="wt0")
            HALF = ND * 128
            # interleave so the first matmul group is gated on ~0.8 MB, not 3.2 MB
            nc.sync.dma_start(x_sb[:, 0:npad], xt[:, 0:npad])
            nc.sync.dma_start(w13_first[:, :HALF], w13[0, :, :HALF])
            for d in range(1, ND):
                nc.sync.dma_start(
                    x_sb[:, d * npad:(d + 1) * npad],
                    xt[:, d * npad:(d + 1) * npad],
                )
            nc.sync.dma_start(w13_first[:, HALF:], w13[0, :, HALF:])
            g_sb = xg_pool.tile([128, NH * npad], F32R, tag="g")

            # stage-2 weight prefetch (filled during stage 1)
            w2_tiles = {}

            def load_w2(dt):
                t = w2_pool.tile([128, NH * 128], F32R, name=f"w2_{dt}", tag="w2")
                nc.sync.dma_start(t[:], w2t[dt])
                w2_tiles[dt] = t

            # ---- stage 1: gT[h, n] = silu(w1.T x)[h, n] * (w3.T x)[h, n] ----
            with tc.tile_pool(name="ps1", bufs=3, space="PSUM") as ps1_pool, \
                 tc.tile_pool(name="ps2", bufs=2, space="PSUM") as ps2_pool:
                for h in range(NH):
                    if h in (10, 16):
                        load_w2({10: 0, 16: 1}[h])
                    if h == 0:
                        wt = w13_first
                    else:
                        wt = w13_pool.tile([128, 2 * ND * 128], F32R, tag="wt")
                        nc.sync.dma_start(wt[:, :HALF], w13[h, :, :HALF])
                        nc.sync.dma_start(wt[:, HALF:], w13[h, :, HALF:])
                    ps = {
                        (ci, s): ps1_pool.tile([128, cl], F32, tag=f"ps{ci}{s}",
                                               name=f"ps_{ci}_{s}")
                        for ci, (cs, cl) in enumerate(chunks) for s in range(2)
                    }
                    # s outermost: h=0's s=1 matmuls start after its DMA half
                    order = [(d, ci, s) for s in range(2)
                             for ci in range(len(chunks)) for d in range(ND)]
                    for d, ci, s in order:
                        cs, cl = chunks[ci]
                        nc.tensor.matmul(
                            ps[ci, s][:],
                            wt[:, (s * ND + d) * 128:(s * ND + d + 1) * 128],
                            x_sb[:, d * npad + cs: d * npad + cs + cl],
                            start=(d == 0),
                            stop=(d == ND - 1),
                        )
                    for ci, (cs, cl) in enumerate(chunks):
                        t_silu = tmp_pool.tile([128, cl], F32, tag=f"silu{ci}")
                        nc.scalar.activation(
                            t_silu[:], ps[ci, 0][:], mybir.ActivationFunctionType.Silu
                        )
                        nc.vector.tensor_mul(
                            g_sb[:, h * npad + cs: h * npad + cs + cl],
                            t_silu[:], ps[ci, 1][:],
                        )

                # ---- stage 2: out[dt, n] = sum_h w2[h, dt].T gT[h, n] ----
                for dt in range(ND):
                    if dt + 2 < ND:
                        load_w2(dt + 2)
                    w2_sb = w2_tiles.pop(dt)
                    for ci, (cs, cl) in enumerate(chunks):
                        ps = ps2_pool.tile([128, cl], F32, tag=f"o{ci}", name="o_ps")
                        for hc in range(NH):
                            nc.tensor.matmul(
                                ps[:],
                                w2_sb[:, hc * 128:(hc + 1) * 128],
                                g_sb[:, hc * npad + cs: hc * npad + cs + cl],
                                start=(hc == 0),
                                stop=(hc == NH - 1),
                            )
                        # split the drain: copy+DMA halves overlap the next MMs
                        half = cl // 2
                        for oi, (ho, hl) in enumerate([(0, half), (half, cl - half)]):
                            o_sb = tmp_pool.tile([128, hl], F32, tag=f"ot{ci}{oi}",
                                                 name="o_sb")
                            nc.scalar.copy(o_sb[:], ps[:, ho:ho + hl])
                            nc.sync.dma_start(
                                outt[dt, :, cs + ho:cs + ho + hl], o_sb[:])
    nc.compile()
    _BUILD_CACHE[key] = nc
    return nc


def _route(expert_indices: np.ndarray):
    """Per-expert token lists, padded count, and an inverse position map."""
    toks = []
    for e in range(E):
        mask = (expert_indices == e).any(axis=1)
        toks.append(np.flatnonzero(mask))
    maxc = max(len(tk) for tk in toks)
    npad = max(512, -(-maxc // 8) * 8)
    inv = np.zeros((E, T), dtype=np.int64)
    for e, tk in enumerate(toks):
        inv[e, tk] = np.arange(len(tk))
    return toks, npad, inv


def _run(inputs, trace=False):
    x = np.ascontiguousarray(inputs["x"], dtype=np.float32)
    idx = np.asarray(inputs["expert_indices"])
    w1 = np.asarray(inputs["w1"], dtype=np.float32)
    w2 = np.asarray(inputs["w2"], dtype=np.float32)
    w3 = np.asarray(inputs["w3"], dtype=np.float32)

    toks, npad, inv = _route(idx)
    nc = _build(npad)

    in_maps = []
    for e in range(E):
        tk = toks[e]
        xg = np.zeros((npad, D), dtype=np.float32)
        xg[: len(tk)] = x[tk]
        # xt[i, d*npad + n] = xg[n, d*128 + i]
        xt = np.ascontiguousarray(
            xg.reshape(npad, ND, 128).transpose(2, 1, 0).reshape(128, ND * npad)
        )
        # w13[h, i, (s*ND + d)*128 + j] = w_s[h*128 + j, d*128 + i]
        w13 = np.stack([w1[e], w3[e]]).reshape(2, NH, 128, ND, 128)
        w13 = np.ascontiguousarray(
            w13.transpose(1, 4, 0, 3, 2).reshape(NH, 128, 2 * ND * 128)
        )
        # w2t[dt, i, hc*128 + j] = w2[hc*128 + i, dt*128 + j]
        w2e = np.ascontiguousarray(
            w2[e].reshape(NH, 128, ND, 128).transpose(2, 1, 0, 3)
            .reshape(ND, 128, NH * 128)
        )
        in_maps.append({"xt": xt, "w13": w13, "w2t": w2e})

    res = run_bass_kernel_spmd(
        nc, in_maps, core_ids=list(range(E)), trace=trace,
        **({"stitch_traces": True} if trace else {}),
    )

    # outs[e, n, dd] = outt[dt, i, n] with dd = dt*128 + i
    outs = np.empty((E, npad, D), dtype=np.float32)
    for e in range(E):
        outs[e] = (
            res.results[e]["outt"].transpose(2, 0, 1).reshape(npad, D)
        )
    final = outs[idx, inv[idx, np.arange(T)[:, None]]]
    return final, res


def kernel(**inputs) -> np.ndarray:
    out, _ = _run(inputs, trace=False)
    return out



# revision 20
# speedup vs baseline: 2.3350x; 2.3350x over previous
"""Expert-parallel MoE ConditionalFeedForward (SwiGLU) for 8 Trainium2 cores.

Math (per token t, selected expert e):
    out[t] = (silu(x[t] @ w1[e].T) * (x[t] @ w3[e].T)) @ w2[e]

Strategy: one expert per NeuronCore (8 experts / 8 cores). The host routes
tokens to experts (gather), each core runs the dense SwiGLU FFN for its
expert's tokens, and the host scatters results back into [T, top_k, D].

Arithmetic: all matmuls run as fp8e4m3 DoubleRow (256-deep contraction per
instruction at half-rate cycles). Precision is recovered with a hi+lo split
of every operand (x, w1, w3, g, w2) and 3-term products per 256-slab:
    a@b ~= a_hi@b_hi + a_lo@b_hi + a_hi@b_lo     (lo@lo dropped, ~0.1% rel)
which lands ~2e-3 relative error vs the fp32 reference (gate is 2e-2).
The hidden activation g = silu(x1)*x3 is scaled by S=1/128 on-chip before
fp8 quantization (e4m3 max is 240; |g| reaches ~2.4e4) and the inverse
scale is applied in the PSUM->SBUF output copy.

Weights fit SBUF entirely in fp8 (17.3 MB -> 135 KB/partition), so all
weight DMAs are issued up-front; startup is fine-grained (per-group /
per-slab-pair) so the first matmul is gated on ~130 KB, not megabytes.
"""

import os

import numpy as np
import ml_dtypes

import concourse.bacc as bacc
import concourse.mybir as mybir
from concourse.bass_utils import run_bass_kernel_spmd
from concourse.tile import TileContext

# Problem constants (nn_ConditionalFeedForward: dim=1024, hidden=2816, 8 experts, top-2)
T = 2048
D = 1024
H = 2816
E = 8
TOPK = 2
ND = D // 128   # 8 d-slabs of 128
NH = H // 128   # 22 h-tiles of 128
S = 1.0 / 128.0  # g pre-scale before fp8 quantization

F32 = mybir.dt.float32
F8 = mybir.dt.float8e4
NPF8 = ml_dtypes.float8_e4m3
DR = mybir.MatmulPerfMode.DoubleRow

_BUILD_CACHE: dict[tuple, object] = {}


def _build(npad: int, loop_n: int = 0):
    """Bass program for one core: dense split-fp8 SwiGLU FFN over npad tokens.

    loop_n > 0 wraps the body in a hardware loop (benchmarking only).
    """
    key = (npad, loop_n)
    if key in _BUILD_CACHE:
        return _BUILD_CACHE[key]
    assert npad % 16 == 0  # DoubleRow moving-slab stride must be 16B-aligned
    # token chunks <= 512 (one PSUM bank of fp32)
    nchunks = -(-npad // 512)
    base = npad // nchunks
    sizes = [base + (1 if i < npad % nchunks else 0) for i in range(nchunks)]
    chunks, off = [], 0
    for sz in sizes:
        chunks.append((off, sz))
        off += sz

    MUL = mybir.AluOpType.mult
    SILU = mybir.ActivationFunctionType.Silu
    COPY = mybir.ActivationFunctionType.Copy

    nc = bacc.Bacc("TRN2", target_bir_lowering=False)
    xh_d = nc.dram_tensor("xh", [128, ND, npad], F8, kind="ExternalInput")
    xl_d = nc.dram_tensor("xl", [128, ND, npad], F8, kind="ExternalInput")
    # per h-tile: 4 groups (w1_hi, w1_lo, w3_hi, w3_lo), each ND slabs x 128 cols
    w13_d = nc.dram_tensor("w13", [NH, 128, 4 * ND, 128], F8, kind="ExternalInput")
    # per d-tile: 2 groups (w2_hi, w2_lo), each NH slabs x 128 cols
    w2_d = nc.dram_tensor("w2q", [ND, 128, 2 * NH, 128], F8, kind="ExternalInput")
    outt = nc.dram_tensor("outt", [ND, 128, npad], F32, kind="ExternalOutput")

    import contextlib

    with TileContext(nc) as tc:
        with (
            tc.For_i(0, loop_n, 1) if loop_n else contextlib.nullcontext(),
            tc.tile_pool(name="big", bufs=1) as big_pool,
            tc.tile_pool(name="tmp", bufs=3) as tmp_pool,
        ):
            xh_sb = big_pool.tile([128, ND, npad], F8, tag="xh", name="xh_sb")
            xl_sb = big_pool.tile([128, ND, npad], F8, tag="xl", name="xl_sb")
            gh_sb = big_pool.tile([128, NH, npad], F8, tag="gh", name="gh_sb")
            gl_sb = big_pool.tile([128, NH, npad], F8, tag="gl", name="gl_sb")
            w13_sb = [
                big_pool.tile([128, 4 * ND, 128], F8, tag=f"w13_{h}", name=f"w13_{h}")
                for h in range(NH)
            ]
            w2_sb = [
                big_pool.tile([128, 2 * NH, 128], F8, tag=f"w2_{d}", name=f"w2_{d}")
                for d in range(ND)
            ]

            # --- DMA issue order: two queues so PE starts early.
            # SP (HWDGE): weights; Pool (SWDGE): x. h=0 weights arrive as 4
            # slab-pair bundles (the host layout groups all 4 weight slices
            # per slab-pair contiguously) matching the pair-major matmul
            # order below.
            def dma_b(q, h, sp):
                q.dma_start(
                    w13_sb[h][:, sp * 8:(sp + 1) * 8, :],
                    w13_d[h, :, sp * 8:(sp + 1) * 8, :],
                )

            def dma_x(q, t, sp):
                dst, src = (xh_sb, xh_d) if t == 0 else (xl_sb, xl_d)
                q.dma_start(
                    dst[:, 2 * sp:2 * sp + 2, :], src[:, 2 * sp:2 * sp + 2, :]
                )

            # SP/HWDGE queue: h0 bundles + 3 x pieces, then bulk weights.
            # Pool/SWDGE queue (gen ~1us each, serial): 5 x pieces.
            # Arrival order tuned so each hi-sweep pair lands just in time.
            dma_b(nc.sync, 0, 0)
            dma_b(nc.sync, 0, 1)
            dma_x(nc.sync, 0, 2)   # xh45
            dma_b(nc.sync, 0, 2)
            dma_b(nc.sync, 0, 3)
            dma_x(nc.sync, 1, 2)   # xl45
            dma_x(nc.sync, 1, 1)   # xl23
            dma_x(nc.sync, 1, 3)   # xl67
            _xq = nc.sync if os.environ.get("KBISECT_NOPOOL") else nc.gpsimd
            _xq.dma_start(xh_sb[:, 0:2, :], xh_d[:, 0:2, :])
            _xq.dma_start(xh_sb[:, 2:4, :], xh_d[:, 2:4, :])
            _xq.dma_start(xh_sb[:, 6:8, :], xh_d[:, 6:8, :])
            _xq.dma_start(xl_sb[:, 0:2, :], xl_d[:, 0:2, :])
            for h in range(1, NH):
                nc.sync.dma_start(w13_sb[h][:], w13_d[h])
            for dt in range(ND):
                nc.sync.dma_start(w2_sb[dt][:], w2_d[dt])

            with tc.tile_pool(name="ps1", bufs=2, space="PSUM") as ps1_pool, \
                 tc.tile_pool(name="ps2", bufs=2, space="PSUM") as ps2_pool, \
                 tc.tile_pool(name="ps2z", bufs=1, space="PSUM") as ps2z_pool:
                # ---- stage 1: g[h, n] = silu(w1.T x)[h, n] * (w3.T x)[h, n] * S
                for h in range(NH):
                    wt = w13_sb[h]
                    for ci, (cs, cl) in enumerate(chunks):
                        pss = [
                            ps1_pool.tile(
                                [128, cl], F32, tag=f"s1_{wi}_{ci}", name=f"ps{wi}_{ci}"
                            )
                            for wi in range(2)  # 0: w1 -> x1, 1: w3 -> x3
                        ]
                        # hi-sweep (4 pairs x [w1hi, w1lo, w3hi, w3lo] vs x_hi)
                        # then lo-sweep (4 pairs x [w1hi, w3hi] vs x_lo), so
                        # startup streams one 128KB bundle + one x piece per
                        # pair and x_lo pieces can trail. g: 0=w1hi 1=w1lo
                        # 2=w3hi 3=w3lo; w1 terms go to pss[0], w3 to pss[1].
                        NSP = ND // 2
                        order = [(sp, g, 0) for sp in range(NSP) for g in range(4)]
                        # lo-sweep pair order matches x_lo piece arrivals
                        order += [(sp, g, 1) for sp in (2, 1, 0, 3) for g in (0, 2)]
                        for sp, g, xi in order:
                            xt = xh_sb if xi == 0 else xl_sb
                            wi = 0 if g < 2 else 1
                            nc.tensor.matmul(
                                pss[wi][:],
                                wt[:, (sp * 4 + g) * 2:(sp * 4 + g) * 2 + 2, :],
                                xt[:, 2 * sp:2 * sp + 2, cs:cs + cl],
                                start=((sp, g) == (0, 0) or (sp, g) == (0, 2))
                                and xi == 0,
                                stop=(sp == NSP - 1 and xi == 1),
                                perf_mode=DR,
                            )
                        t_silu = tmp_pool.tile([128, cl], F32, tag=f"silu{ci}")
                        nc.scalar.activation(
                            t_silu[:], pss[0][:],
                            COPY if os.environ.get("KBISECT_NOSILU") else SILU,
                        )
                        g_s = tmp_pool.tile([128, cl], F32, tag=f"gs{ci}")
                        # g_s = (silu(x1) * S) * x3
                        nc.vector.scalar_tensor_tensor(
                            g_s[:], t_silu[:], float(S), pss[1][:], MUL, MUL
                        )
                        nc.scalar.activation(gh_sb[:, h, cs:cs + cl], g_s[:], COPY)
                        nc.vector.tensor_sub(
                            gl_sb[:, h, cs:cs + cl], g_s[:], gh_sb[:, h, cs:cs + cl]
                        )

                # ---- stage 2: out[dt, n] = sum_h w2[h, dt].T g[h, n] / S ----
                for dt in range(ND):
                    w2t = w2_sb[dt]
                    # the very last (dt, chunk) is split into a big + a small
                    # token piece so the final drain chain rides on 128 tokens
                    sub = list(chunks)
                    if dt == ND - 1 and not os.environ.get("KBISECT_NOZ"):
                        cs, cl = sub.pop()
                        if cl > 128:
                            sub += [(cs, cl - 128, "z0"), (cs + cl - 128, 128, "z1")]
                        else:
                            sub.append((cs, cl, "z0"))
                    sub = [c if len(c) == 3 else (c[0], c[1], None) for c in sub]
                    for ci, (cs, cl, zt) in enumerate(sub):
                        tag = f"o{zt}" if zt else f"o{ci}"
                        pool = ps2z_pool if zt else ps2_pool
                        ps = pool.tile([128, cl], F32, tag=tag, name="o_ps")
                        nmm = 3 * (NH // 2)
                        k = 0
                        for kp in range(0, NH, 2):  # kp outer: last g tiles last
                            # terms: (w2_hi, g_hi), (w2_hi, g_lo), (w2_lo, g_hi)
                            for s2 in range(3):
                                w2off = 0 if s2 < 2 else NH
                                gt = gl_sb if s2 == 1 else gh_sb
                                nc.tensor.matmul(
                                    ps[:],
                                    w2t[:, w2off + kp:w2off + kp + 2, :],
                                    gt[:, kp:kp + 2, cs:cs + cl],
                                    start=(k == 0),
                                    stop=(k == nmm - 1),
                                    perf_mode=DR,
                                )
                                k += 1
                        # drain: one scaled copy + DMA per chunk (hides under
                        # the next chunk's matmuls; only the last z1 chain is
                        # exposed, and it rides on 128 tokens)
                        o_sb = tmp_pool.tile([128, cl], F32, tag=f"ot{tag}")
                        nc.scalar.activation(
                            o_sb[:], ps[:], COPY, scale=float(1.0 / S)
                        )
                        nc.sync.dma_start(outt[dt, :, cs:cs + cl], o_sb[:])
    nc.compile()
    _BUILD_CACHE[key] = nc
    return nc


def _route(expert_indices: np.ndarray):
    """Per-expert token lists, padded count, and an inverse position map."""
    toks = []
    for e in range(E):
        mask = (expert_indices == e).any(axis=1)
        toks.append(np.flatnonzero(mask))
    maxc = max(len(tk) for tk in toks)
    npad = -(-maxc // 16) * 16
    inv = np.zeros((E, T), dtype=np.int64)
    for e, tk in enumerate(toks):
        inv[e, tk] = np.arange(len(tk))
    return toks, npad, inv


def _split8(a: np.ndarray):
    """fp8e4m3 hi + lo residual (both fp8)."""
    hi = np.asarray(a, dtype=NPF8)
    lo = np.asarray(a - hi.astype(np.float32), dtype=NPF8)
    return hi, lo


def _prep_in_maps(inputs):
    """Host-side routing, quantization, and layout. Returns (in_maps, npad, idx, inv)."""
    x = np.ascontiguousarray(inputs["x"], dtype=np.float32)
    idx = np.asarray(inputs["expert_indices"])
    w1 = np.asarray(inputs["w1"], dtype=np.float32)
    w2 = np.asarray(inputs["w2"], dtype=np.float32)
    w3 = np.asarray(inputs["w3"], dtype=np.float32)

    toks, npad, inv = _route(idx)
    xq_hi, xq_lo = _split8(x)  # [T, D]

    in_maps = []
    for e in range(E):
        tk = toks[e]

        def xlay(xq):
            xg = np.zeros((npad, D), dtype=NPF8)
            xg[: len(tk)] = xq[tk]
            # [128 i, ND d, npad n] with element (n, d*128+i)
            return np.ascontiguousarray(
                xg.reshape(npad, ND, 128).transpose(2, 1, 0)
            )

        h1, l1 = _split8(w1[e])
        h3, l3 = _split8(w3[e])
        grp = np.stack([h1, l1, h3, l3])               # [4, H, D]
        # bundle layout: [ht, i, (sp, g, p), j] with d-slab = sp*2 + p, so one
        # slab-pair's 4 weight slices are contiguous (startup DMA bundles)
        grp = grp.reshape(4, NH, 128, ND // 2, 2, 128)  # [g, ht, j, sp, p, i]
        w13 = np.ascontiguousarray(grp.transpose(1, 5, 3, 0, 4, 2))  # [ht,i,sp,g,p,j]
        w13 = w13.reshape(NH, 128, 4 * ND, 128)

        h2, l2 = _split8(w2[e])
        g2 = np.stack([h2, l2])                        # [s, H, D]
        g2 = g2.reshape(2, NH, 128, ND, 128)           # [s, k, i, dt, j]
        w2q = np.ascontiguousarray(g2.transpose(3, 2, 0, 1, 4))  # [dt, i, s, k, j]
        w2q = w2q.reshape(ND, 128, 2 * NH, 128)

        in_maps.append(
            {"xh": xlay(xq_hi), "xl": xlay(xq_lo), "w13": w13, "w2q": w2q}
        )
    return in_maps, npad, idx, inv


def _run(inputs, trace=False):
    in_maps, npad, idx, inv = _prep_in_maps(inputs)
    nc = _build(npad)

    res = run_bass_kernel_spmd(
        nc, in_maps, core_ids=list(range(E)), trace=trace,
        **({"stitch_traces": True} if trace else {}),
    )

    # outs[e, n, dd] = outt[dt, i, n] with dd = dt*128 + i
    outs = np.empty((E, npad, D), dtype=np.float32)
    for e in range(E):
        outs[e] = (
            res.results[e]["outt"].transpose(2, 0, 1).reshape(npad, D)
        )
    final = outs[idx, inv[idx, np.arange(T)[:, None]]]
    return final, res


def kernel(**inputs) -> np.ndarray:
    out, _ = _run(inputs, trace=False)
    return out
